# revision 1
# baseline (speedup 1.0000x reference)
"""Trainium2 Bass kernel for nn_AnnsHNSW (retrieval kNN + anns pairing), v2.

Full inputs: query [2,16,2048,64] f32, key [2,16,2048,64] f32, sample_size=64.
Output: (query_sort_idx [2,16,2048] i32, key_pick_idx [2,16,2048] i32).

Math note: the reference's QNF augmentation adds |k_aug|^2 == kmax^2 (a
constant) to every key, and scales each query by r_q > 0.  Both are
order-preserving per query, so the kNN ordering equals ordering by the plain
inner product q.k (descending).  Verified on the reference data: 0 qsi
mismatches, 8 kpi tie-artifact mismatches (relerr 6.3e-3 << 2e-2).

Per core (4 slices): labels via argmax of fp32 q.k (PE matmul -> PSUM,
DVE Max/MaxIndex over two 1024-wide halves), stable argsort via the
count-less-than trick (ACT Sign+accum over a broadcast row of combined keys
c = 2048*label + qidx), qsi written with gpsimd indirect scatters.  The
picked queries (rank % 64 == 0) are recovered ON-CORE with a one-hot
matmul (A_t[p,p'] = [rank==64p'], picked = sum_t A_t^T qiota_t), so nothing
waits on the qsi scatters; their top-64 comes from 8 rounds of
Max8/MaxIndex/MatchReplace while the last scatters drain.

The slice pipeline overlaps: labels(s) runs while rank/scatter/picked(s-1)
and preproc(s+1) proceed on ACT/Pool/PE/DMA.
"""

import os

import numpy as np

B, H, NQ, NK, D = 2, 16, 2048, 2048, 64
SAMPLE = 64
N_CORES = 8
SL = (B * H) // N_CORES  # slices per core

NEG_BIG = -1.0e30


def build_bass(n_slices=SL, nq=NQ, nk=NK, d=D, sample=SAMPLE, split_waits=True,
               debug=False):
    import concourse.bass as bass
    import concourse.mybir as mybir
    from concourse.tile import TileContext
    from concourse.masks import make_identity

    f32 = mybir.dt.float32
    i32 = mybir.dt.int32
    u32 = mybir.dt.uint32
    AF = mybir.ActivationFunctionType
    ALU = mybir.AluOpType
    AX = mybir.AxisListType

    nqt = nq // 128          # q tiles per slice (16)
    nkt = nk // 128          # k tiles per slice (16)
    half = nk // 2           # 1024
    npick = nq // sample     # picked queries per slice (32)
    nused = npick * n_slices
    assert nused <= 128
    N_DVE_TAIL_COLS = 5      # tail rank columns handled by DVE (rest on ACT)

    nc = bass.Bass()
    q_in = nc.declare_dram_parameter("query", [n_slices * nq, d], f32, isOutput=False)
    k_in = nc.declare_dram_parameter("key", [n_slices, nk, d], f32, isOutput=False)
    qiota_f_in = nc.declare_dram_parameter("qiota_f", [128, nqt], f32, isOutput=False)
    qiota_i_in = nc.declare_dram_parameter("qiota_i", [128, nqt], i32, isOutput=False)
    c64_in = nc.declare_dram_parameter("c64", [128, npick], f32, isOutput=False)
    qsi_out = nc.declare_dram_parameter("qsi", [n_slices * nq, 1], i32, isOutput=True)
    kpi_out = nc.declare_dram_parameter("kpi", [n_slices, nq], i32, isOutput=True)

    if debug:
        dbg_lab = nc.declare_dram_parameter("dbg_lab", [n_slices, 128, nqt], f32, isOutput=True)
        dbg_rank = nc.declare_dram_parameter("dbg_rank", [n_slices, 128, nqt], f32, isOutput=True)
        dbg_pick = nc.declare_dram_parameter("dbg_pick", [n_slices, npick], i32, isOutput=True)

    crow_dram = nc.dram_tensor("crow_dram", [n_slices, nq], f32)

    with TileContext(nc) as tc:
        with (
            tc.tile_pool(name="const", bufs=1) as constp,
            tc.tile_pool(name="kbigp", bufs=2) as kbigp,
            tc.tile_pool(name="qbigp", bufs=2) as qbigp,
            tc.tile_pool(name="ktp", bufs=n_slices) as ktp,
            tc.tile_pool(name="qtp", bufs=2) as qtp,
            tc.tile_pool(name="scanp", bufs=2) as scanp,
            tc.tile_pool(name="smallp", bufs=2) as smallp,
            tc.tile_pool(name="cbp", bufs=2) as cbp,
            tc.tile_pool(name="scrp", bufs=2) as scrp,
            tc.tile_pool(name="pickp", bufs=2) as pickp,
            tc.tile_pool(name="finalp", bufs=1) as finalp,
            tc.tile_pool(name="v8p", bufs=4) as v8p,
            tc.tile_pool(name="ps_scA", bufs=1, space="PSUM") as ps_scAp,
            tc.tile_pool(name="ps_scB", bufs=1, space="PSUM") as ps_scBp,
            tc.tile_pool(name="ps_trp", bufs=2, space="PSUM") as ps_trp,
            tc.tile_pool(name="ps_pkp", bufs=1, space="PSUM") as ps_pkp,
            tc.tile_pool(name="ps_ppp", bufs=1, space="PSUM") as ps_ppp,
        ):
            # ---- constants ----
            ident = constp.tile([128, 128], f32, tag="ident")
            make_identity(nc, ident[:])
            ones1 = constp.tile([1, 128], f32, tag="ones1")
            nc.vector.memset(ones1[:], 1.0)
            qiota_f = constp.tile([128, nqt], f32, tag="qiota_f")
            nc.sync.dma_start(qiota_f[:], qiota_f_in[:])
            qiota_i = constp.tile([128, nqt], i32, tag="qiota_i")
            nc.sync.dma_start(qiota_i[:], qiota_i_in[:])
            c64 = constp.tile([128, npick], f32, tag="c64")
            nc.sync.dma_start(c64[:], c64_in[:])

            # persistent PSUM
            psA = ps_scAp.tile([128, half], f32, tag="psA")
            psB = ps_scBp.tile([128, half], f32, tag="psB")
            ps_pp = ps_ppp.tile([nused, 512], f32, tag="ps_pp")

            # picked-phase persistent tiles
            pqt = finalp.tile([d, nused], f32, tag="pqt")
            psc = finalp.tile([nused, nk], f32, tag="psc")
            topidx = finalp.tile([nused, sample], i32, tag="topidx")

            # warmups: dummy PE matmul (absorbs ident's gpsimd sem) and an
            # ACT Sign op (loads the act table before the critical path)
            wtr = ps_trp.tile([128, 128], f32, tag="ps_tr")
            nc.tensor.matmul(wtr[:], lhsT=ident[:], rhs=ident[:], start=True, stop=True)
            dscrap = constp.tile([1, 1], f32, tag="dscrap")
            nc.vector.tensor_copy(dscrap[:], wtr[0:1, 0:1])
            wsig = constp.tile([1, 1], f32, tag="wsig")
            nc.scalar.activation(wsig[:], ones1[0:1, 0:1], AF.Sign, bias=0.0, scale=1.0)

            kts = {}
            qts = {}
            kbigs = {}
            qbigs = {}
            va8s = {}
            vb8s = {}
            ia8s = {}
            ib8s = {}
            cs = {}
            accs = {}
            rankfs = {}
            rankis = {}
            cbs = {}
            pickis = {}
            pqs = {}

            def load_kq(s):
                if s >= n_slices:
                    return
                kbig = kbigp.tile([128, nkt * d], f32, tag="kbig")
                kbigs[s] = kbig
                nc.sync.dma_start(
                    kbig[:].rearrange("p (t d) -> p t d", d=d),
                    k_in[s].rearrange("(t p) d -> p t d", p=128),
                )
                qbig = qbigp.tile([128, nqt * d], f32, tag="qbig")
                qbigs[s] = qbig
                nc.sync.dma_start(
                    qbig[:].rearrange("p (t d) -> p t d", d=d),
                    q_in[:].rearrange("(s t p) d -> s p t d", s=n_slices, p=128)[s],
                )

            def alloc_kq(s):
                kts[s] = ktp.tile([d, nk], f32, tag="kt", name="kt")
                qts[s] = qtp.tile([d, nq], f32, tag="qt", name="qt")

            def preproc_group(s, j0, cnt):
                """Transpose k/q tiles j0..j0+cnt-1 of slice s into kt/qt."""
                if s >= n_slices:
                    return
                kt, qt = kts[s], qts[s]
                kbig, qbig = kbigs[s], qbigs[s]
                for j in range(j0, j0 + cnt):
                    ptr = ps_trp.tile([128, 128], f32, tag="ps_tr")
                    nc.tensor.transpose(ptr[0:d, :], kbig[:, j * d:(j + 1) * d], ident[:])
                    nc.scalar.copy(kt[0:d, j * 128:(j + 1) * 128], ptr[0:d, :])
                    ptr2 = ps_trp.tile([128, 128], f32, tag="ps_tr")
                    nc.tensor.transpose(ptr2[0:d, :], qbig[:, j * d:(j + 1) * d], ident[:])
                    nc.scalar.copy(qt[0:d, j * 128:(j + 1) * 128], ptr2[0:d, :])

            def alloc_scan(s):
                va8s[s] = scanp.tile([128, 8 * nqt], f32, tag="va8", name="va8")
                vb8s[s] = scanp.tile([128, 8 * nqt], f32, tag="vb8", name="vb8")
                ia8s[s] = scanp.tile([128, 8 * nqt], u32, tag="ia8", name="ia8")
                ib8s[s] = scanp.tile([128, 8 * nqt], u32, tag="ib8", name="ib8")

            def scores(s, t):
                kt, qt = kts[s], qts[s]
                lhs = qt[:, t * 128:(t + 1) * 128]
                nc.tensor.matmul(psA[:, 0:512], lhsT=lhs, rhs=kt[:, 0:512],
                                 start=True, stop=True)
                nc.tensor.matmul(psA[:, 512:1024], lhsT=lhs, rhs=kt[:, 512:1024],
                                 start=True, stop=True)
                nc.tensor.matmul(psB[:, 0:512], lhsT=lhs, rhs=kt[:, half:half + 512],
                                 start=True, stop=True)
                nc.tensor.matmul(psB[:, 512:1024], lhsT=lhs, rhs=kt[:, half + 512:nk],
                                 start=True, stop=True)

            def scans(s, t):
                va8, vb8, ia8, ib8 = va8s[s], vb8s[s], ia8s[s], ib8s[s]
                va = va8[:, t * 8:(t + 1) * 8]
                nc.vector.max(out=va, in_=psA[:])
                nc.vector.max_index(out=ia8[:, t * 8:(t + 1) * 8], in_max=va, in_values=psA[:])
                vb = vb8[:, t * 8:(t + 1) * 8]
                nc.vector.max(out=vb, in_=psB[:])
                nc.vector.max_index(out=ib8[:, t * 8:(t + 1) * 8], in_max=vb, in_values=psB[:])

            def merge(s):
                """labels + combined sort key c for slice s (small DVE ops)."""
                va8, vb8, ia8, ib8 = va8s[s], vb8s[s], ia8s[s], ib8s[s]
                vaS = va8[:].rearrange("p (t e) -> p t e", e=8)[:, :, 0]
                vbS = vb8[:].rearrange("p (t e) -> p t e", e=8)[:, :, 0]
                iaS = ia8[:].rearrange("p (t e) -> p t e", e=8)[:, :, 0]
                ibS = ib8[:].rearrange("p (t e) -> p t e", e=8)[:, :, 0]
                iaf = smallp.tile([128, nqt], f32, tag="iaf")
                nc.vector.tensor_copy(iaf[:], iaS)
                ibo = smallp.tile([128, nqt], f32, tag="ibo")
                nc.vector.tensor_copy(ibo[:], ibS)
                nc.vector.tensor_scalar(ibo[:], ibo[:], float(half), None, op0=ALU.add)
                ge = smallp.tile([128, nqt], f32, tag="ge")
                nc.vector.tensor_tensor(ge[:], vaS, vbS, op=ALU.is_ge)
                # labf = ge ? iaf : ibo   (= ibo - ge*(ibo - iaf))
                dm = smallp.tile([128, nqt], f32, tag="dm")
                nc.vector.tensor_tensor(dm[:], ibo[:], iaf[:], op=ALU.subtract)
                nc.vector.tensor_tensor(dm[:], dm[:], ge[:], op=ALU.mult)
                labf = smallp.tile([128, nqt], f32, tag="labf")
                nc.vector.tensor_tensor(labf[:], ibo[:], dm[:], op=ALU.subtract)
                c = smallp.tile([128, nqt], f32, tag="c")
                nc.vector.tensor_scalar(c[:], labf[:], float(nq), None, op0=ALU.mult)
                nc.vector.tensor_tensor(c[:], c[:], qiota_f[:], op=ALU.add)
                cs[s] = c
                if debug:
                    nc.sync.dma_start(dbg_lab[s], labf[:])

            def rank_tr(s):
                """c [128,16] -> crow_dram row (PE transpose + store)."""
                c = cs[s]
                ptr = ps_trp.tile([128, 128], f32, tag="ps_tr")
                nc.tensor.transpose(ptr[0:nqt, :], c[:], ident[:])
                ct = smallp.tile([nqt, 128], f32, tag="ct")
                nc.scalar.copy(ct[:], ptr[0:nqt, :])
                nc.sync.dma_start(
                    crow_dram[s].rearrange("(t p) -> t p", t=nqt), ct[:]
                )

            def cb_build(s):
                """cb [128,2048] = crow_dram row broadcast (one DMA)."""
                cb = cbp.tile([128, nq], f32, tag="cb")
                nc.sync.dma_start(cb[:], crow_dram[s:s + 1, :].broadcast_to([128, nq]))
                accs[s] = smallp.tile([128, nqt], f32, tag="acc", name="acc")
                cbs[s] = cb

            def rank_cols(s, t0, cnt=2):
                """ACT Sign+accum rank columns t0..t0+cnt-1 of slice s."""
                c, acc, cb = cs[s], accs[s], cbs[s]
                for t in range(t0, t0 + cnt):
                    rscr = scrp.tile([128, nq], f32, tag="rscr")
                    nc.scalar.activation(rscr[:], cb[:], AF.Sign,
                                         bias=c[:, t:t + 1], scale=-1.0,
                                         accum_out=acc[:, t:t + 1])

            def rank_fin(s):
                acc = accs[s]
                rankf = smallp.tile([128, nqt], f32, tag="rankf")
                nc.vector.tensor_scalar(rankf[:], acc[:], float(nq - 1), 0.5,
                                        op0=ALU.add, op1=ALU.mult)
                ranki = smallp.tile([128, nqt], i32, tag="ranki")
                nc.vector.tensor_copy(ranki[:], rankf[:])
                rankfs[s] = rankf
                rankis[s] = ranki
                if debug:
                    nc.sync.dma_start(dbg_rank[s], rankf[:])

            def picked_extract_col(s, t, ps_pk):
                """One-hot A_t = [rank_t == 64p'] and accumulate A_t^T qiota_t."""
                rankf = rankfs[s]
                A = smallp.tile([128, npick], f32, tag="Aoh")
                nc.vector.tensor_tensor(
                    A[:], rankf[:, t:t + 1].broadcast_to([128, npick]), c64[:],
                    op=ALU.is_equal)
                nc.tensor.matmul(ps_pk[:], lhsT=A[:], rhs=qiota_f[:, t:t + 1],
                                 start=(t == 0), stop=(t == nqt - 1))

            def picked_fin(s, ps_pk):
                picki = pickp.tile([npick, 1], i32, tag="picki")
                nc.vector.tensor_copy(picki[:], ps_pk[:])
                pickis[s] = picki
                if debug:
                    nc.sync.dma_start(dbg_pick[s].rearrange("j -> j ()"), picki[:])

            def picked_extract(s):
                ps_pk = ps_pkp.tile([npick, 1], f32, tag="ps_pk")
                for t in range(nqt):
                    picked_extract_col(s, t, ps_pk)
                picked_fin(s, ps_pk)

            def picked_gather(s):
                pq = pickp.tile([npick, d], f32, tag="pq")
                pqs[s] = pq
                nc.gpsimd.indirect_dma_start(
                    out=pq[:],
                    out_offset=None,
                    in_=q_in[0:npick, :],
                    in_offset=bass.IndirectOffsetOnAxis(ap=pickis[s][:], axis=0),
                    element_offset=s * nq * d,
                )

            def scatters(s, t0=0, cnt=nqt):
                ranki = rankis[s]
                for t in range(t0, t0 + cnt):
                    # out AP window kept small: SWDGE descriptor count (and the
                    # cost model) size by the declared AP, not the 128 writes
                    nc.gpsimd.indirect_dma_start(
                        out=qsi_out[0:128, :],
                        out_offset=bass.IndirectOffsetOnAxis(
                            ap=ranki[:, t:t + 1], axis=0),
                        in_=qiota_i[:, t:t + 1],
                        in_offset=None,
                        element_offset=s * nq,
                    )

            def picked_scores(s):
                pq = pqs[s]
                ptr = ps_trp.tile([128, 128], f32, tag="ps_tr")
                nc.tensor.transpose(ptr[0:d, 0:npick], pq[:], ident[0:npick, 0:npick])
                nc.scalar.copy(pqt[0:d, s * npick:(s + 1) * npick], ptr[0:d, 0:npick])
                for n in range(nk // 512):
                    nc.tensor.matmul(
                        ps_pp[s * npick:(s + 1) * npick, :],
                        lhsT=pqt[:, s * npick:(s + 1) * npick],
                        rhs=kts[s][:, n * 512:(n + 1) * 512],
                        start=True, stop=True,
                        tile_position=(0, s * npick),
                    )
                    nc.scalar.copy(
                        psc[s * npick:(s + 1) * npick, n * 512:(n + 1) * 512],
                        ps_pp[s * npick:(s + 1) * npick, :],
                    )

            # ================= startup =================
            load_kq(0)
            alloc_kq(0)
            preproc_group(0, 0, nqt)

            # ================= main pipeline =================
            for s in range(n_slices):
                alloc_scan(s)
                if s + 1 < n_slices:
                    alloc_kq(s + 1)
                for t in range(nqt):
                    scores(s, t)
                    # hooks (program-order placement between PE scores and
                    # DVE scans keeps other engines fed without stalling them)
                    if t == 0:
                        load_kq(s + 1)
                        if s >= 1:
                            merge(s - 1)
                    elif t == 1 and s >= 1:
                        rank_tr(s - 1)
                    elif t == 3 and s >= 1:
                        cb_build(s - 1)
                    elif t == 12 and s >= 1:
                        rank_fin(s - 1)
                        picked_extract(s - 1)
                    elif t == 13 and s >= 1:
                        picked_gather(s - 1)   # Pool: before the scatters
                        scatters(s - 1)
                    elif t == 15 and s >= 1:
                        picked_scores(s - 1)
                    if 4 <= t <= 11 and s >= 1:
                        rank_cols(s - 1, (t - 4) * 2)
                    if 4 <= t <= 11 and s + 1 < n_slices:
                        preproc_group(s + 1, (t - 4) * 2, 2)
                    scans(s, t)

            # ================= tail: slice n-1 rank + picked =================
            sl = n_slices - 1
            merge(sl)
            rank_tr(sl)
            cb_build(sl)
            # rank columns for the last slice: first N_DVE_TAIL_COLS on DVE
            # (idle in the tail), the rest on ACT; per-column ranki + scatter
            # interleave so the scatter queue drains alongside
            c, acc, cb = cs[sl], accs[sl], cbs[sl]
            ranki = smallp.tile([128, nqt], i32, tag="ranki")
            rankis[sl] = ranki
            rankf = smallp.tile([128, nqt], f32, tag="rankf")
            rankfs[sl] = rankf
            ps_pk = ps_pkp.tile([npick, 1], f32, tag="ps_pk")
            for t in range(nqt):
                if t < N_DVE_TAIL_COLS:
                    tmplt = scrp.tile([128, nq], f32, tag="tmplt")
                    nc.vector.tensor_scalar(tmplt[:], cb[:], c[:, t:t + 1], None,
                                            op0=ALU.is_lt)
                    nc.vector.reduce_sum(rankf[:, t:t + 1], tmplt[:], axis=AX.X)
                else:
                    rscr = scrp.tile([128, nq], f32, tag="rscr")
                    nc.scalar.activation(rscr[:], cb[:], AF.Sign,
                                         bias=c[:, t:t + 1], scale=-1.0,
                                         accum_out=acc[:, t:t + 1])
                    nc.vector.tensor_scalar(rankf[:, t:t + 1], acc[:, t:t + 1],
                                            float(nq - 1), 0.5, op0=ALU.add, op1=ALU.mult)
                nc.vector.tensor_copy(ranki[:, t:t + 1], rankf[:, t:t + 1])
                picked_extract_col(sl, t, ps_pk)
            if debug:
                nc.sync.dma_start(dbg_rank[sl], rankf[:])
            picked_fin(sl, ps_pk)
            picked_gather(sl)      # Pool: ahead of the final scatters
            scatters(sl)           # drain concurrently with extraction
            picked_scores(sl)

            # ================= top-64 extraction =================
            for r in range(sample // 8):
                pv8 = v8p.tile([nused, 8], f32, tag="pv8")
                nc.vector.max(out=pv8[:], in_=psc[:])
                nc.vector.max_index(
                    out=topidx[:, r * 8:(r + 1) * 8].bitcast(u32),
                    in_max=pv8[:], in_values=psc[:],
                )
                if r < sample // 8 - 1:
                    nc.vector.match_replace(
                        out=psc[:], in_to_replace=pv8[:], in_values=psc[:],
                        imm_value=NEG_BIG,
                    )

            for s in range(n_slices):
                nc.sync.dma_start(
                    kpi_out[s].rearrange("(j k) -> j k", k=sample),
                    topidx[s * npick:(s + 1) * npick, :],
                )

    if split_waits:
        import concourse.mybir as mybir_mod
        _split_multi_waits(nc, mybir_mod)
    return nc


def _split_multi_waits(nc, mybir):
    """Walrus accepts only ONE sync-wait per instruction; move extras onto
    same-engine NoOps inserted before the offending instruction."""
    n = 0
    for f in nc.m.functions:
        for blk in f.blocks:
            out = []
            for inst in blk.instructions:
                si = getattr(inst, "sync_info", None)
                if si is not None and len(si.on_wait) > 1:
                    waits = list(si.on_wait)
                    for w in waits[:-1]:
                        nop = mybir.InstNoOp(
                            name=f"I-wsplit-{n}", ins=[], outs=[],
                            text_hint="wsplit",
                        )
                        n += 1
                        nop.engine = inst.engine
                        nop.sync_info = mybir.SyncInfo(on_wait=[w], on_update=[])
                        out.append(nop)
                    inst.sync_info = mybir.SyncInfo(
                        on_wait=[waits[-1]], on_update=list(si.on_update)
                    )
                out.append(inst)
            blk.instructions = out
    return nc


_BUILT = {}
LAST_RESULTS = None


def _get_nc(key=(SL, NQ, NK, D, SAMPLE)):
    if key not in _BUILT:
        _BUILT[key] = build_bass(*key)
    return _BUILT[key]


def make_iota(nqt=NQ // 128):
    # qiota[p, t] = t*128 + p
    p = np.arange(128, dtype=np.int64)[:, None]
    t = np.arange(nqt, dtype=np.int64)[None, :]
    v = (t * 128 + p)
    return v.astype(np.float32), v.astype(np.int32)


def kernel(query, key, sample_size=SAMPLE):
    from concourse.bass_utils import run_bass_kernel_spmd

    q = np.ascontiguousarray(np.asarray(query, dtype=np.float32)).reshape(B * H, NQ, D)
    k = np.ascontiguousarray(np.asarray(key, dtype=np.float32)).reshape(B * H, NK, D)
    iota_f, iota_i = make_iota()
    c64 = np.broadcast_to(
        (np.arange(NQ // SAMPLE, dtype=np.float32) * SAMPLE)[None, :], (128, NQ // SAMPLE)
    ).copy()

    in_maps = []
    for c in range(N_CORES):
        in_maps.append(
            {
                "query": np.ascontiguousarray(
                    q[c * SL:(c + 1) * SL].reshape(SL * NQ, D)),
                "key": np.ascontiguousarray(k[c * SL:(c + 1) * SL]),
                "qiota_f": iota_f,
                "qiota_i": iota_i,
                "c64": c64,
            }
        )

    nc = _get_nc()
    trace = bool(os.environ.get("ANNS_TRACE"))
    res = run_bass_kernel_spmd(
        nc, in_maps, core_ids=list(range(N_CORES)), trace=trace
    )
    global LAST_RESULTS
    LAST_RESULTS = res
    qsi = np.concatenate(
        [res.results[i]["qsi"].reshape(SL, NQ) for i in range(N_CORES)], axis=0
    ).reshape(B, H, NQ)
    kpi = np.concatenate(
        [res.results[i]["kpi"].reshape(SL, NQ) for i in range(N_CORES)], axis=0
    ).reshape(B, H, NQ)
    return qsi.astype(np.int32), kpi.astype(np.int32)


if __name__ == "__main__":
    rng = np.random.default_rng(0)
    q = rng.normal(size=(B, H, NQ, D)).astype(np.float32)
    k = rng.normal(size=(B, H, NK, D)).astype(np.float32)
    out = kernel(q, k, SAMPLE)
    print([o.shape for o in out])

